# revision 1
# baseline (speedup 1.0000x reference)
"""Trainium2 Bass kernel for nn_Correlation (FlowNet-style cost volume).

Problem: input1/input2 [8, 256, 96, 128] f32 ->
         out [8, 441, 96, 128] f32
  out[b, 21*i+j, h, w] = leaky_relu_0.1( (1/256) * sum_c
        in1[b,c,h,w] * in2pad[b,c, h+2i, w+2j] )       (pad 20 each side)

Strategy (data-parallel over B across 8 cores; per core = 1 sample):
  * Displacements are even (dilation 2): pixel (h,w) only correlates with
    in2 pixels of the same (h%2, w%2) parity class. In parity space the
    dilated 21x21 patch is a dense 21x21 window.
  * Per parity class: split the 48x64 parity image into 8x16 pixel
    blocks (128 pixels = PE stationary operand). Stream the block's
    28x36 in2 parity window through the PE contracting over C=256
    (bf16, fp32 PSUM) -> band[pixel, window_col] (1008 cols, 441 useful).
  * PE operands need single-stride APs, so both inputs are rearranged
    on-chip (GPSIMD copies): in1 into parity-blocked contiguous 128-pixel
    groups; in2 into per-wb "bands" where each block's window rows are
    contiguous (36-row rolling buffer over padded parity rows).
  * Band -> SBUF (DVE) -> DRAM scratch (contiguous) -> diagonal gather
    back (per-pixel 21x21 patch; DRAM-side APs are flat so the diagonal
    is legal) -> ALIGNED[pixel, 441].
  * PE-transpose ALIGNED -> [d, pixel]; ScalarE applies
    leaky_relu(x/256) scattering into parity-interleaved row tiles;
    contiguous stores.
"""

import numpy as np

import concourse.bass as bass
import concourse.mybir as mybir
from concourse.tile import TileContext
from concourse.bass_utils import run_bass_kernel_spmd
from concourse.masks import make_identity

DT = mybir.dt

# ---- problem geometry ----
B, C, H, W = 8, 256, 96, 128
NP = 21                      # displacements per axis
ND = NP * NP                 # 441
CC = 2                       # C chunks of 128
HE, WE = H // 2, W // 2      # parity image 48 x 64
HEP, WEP = HE + 20, WE + 20  # padded parity image 68 x 84

HB, WB = 8, 16               # parity block (he, we); HB*WB = 128
WIN_H, WIN_W = HB + 20, WB + 20   # 28 x 36 window
FB = WIN_H * WIN_W           # 1008 band columns
HW = H * W                   # 12288

# in2 "bands": per (cc, hp, wp, wb) a [NSLOT, 36] contiguous-row image,
# rolling over padded parity rows (h'e in [0, 68), slot = h'e % NSLOT).
# NSLOT=40 (not 36) so a prefetched group only overwrites rows of
# ALREADY-FINISHED block-rows (dependency slack for pipelining).
NSLOT = 40
BAND_PITCH = NSLOT * WIN_W               # 1296
N_BANDS = CC * 2 * 2 * 4                 # 32
BANDS_F = N_BANDS * BAND_PITCH           # 41472

IN1BLK_F = 2 * 2 * CC * 4 * 128          # 4096 per block-row slab
STG_F = CC * 16 * W                      # 4096 (16 full-res rows)

_MAX_WAITS = 1


def _split_excess_waits(nc):
    """This walrus build accepts only ONE sync-wait per instruction; Tile
    emits multi-waits. Hoist excess waits onto same-engine NOPs inserted
    right before the over-subscribed instruction."""
    nid = 0
    for f in nc.m.functions:
        for blk in f.blocks:
            insts = list(blk.instructions)
            out = []
            changed = False
            for inst in insts:
                si = inst.sync_info
                if si is not None and si.on_wait and len(si.on_wait) > _MAX_WAITS:
                    waits = list(si.on_wait)
                    extra, keep = waits[:-_MAX_WAITS], waits[-_MAX_WAITS:]
                    for k in range(0, len(extra), _MAX_WAITS):
                        nop = mybir.InstNoOp(name=f"I-waitsplit-{nid}", ins=[], outs=[])
                        nid += 1
                        nop.engine = inst.engine
                        nop.sync_info = mybir.SyncInfo(
                            on_wait=extra[k : k + _MAX_WAITS], on_update=[]
                        )
                        out.append(nop)
                        changed = True
                    si.on_wait = keep
                    inst.sync_info = si
                out.append(inst)
            if changed:
                blk.instructions = out
    return nc


def _ap(t, off_extra, dims):
    return bass.AP(tensor=t.tensor, offset=t.offset + off_extra, ap=dims)


def _band_base(cc, hp, wp, wb):
    return (((cc * 2 + hp) * 2 + wp) * 4 + wb) * BAND_PITCH


def _slot_runs(lo, hi):
    """Contiguous (slot, h'e, count) runs for padded parity rows [lo, hi)."""
    runs = []
    r = lo
    while r < hi:
        s = r % NSLOT
        n = min(hi - r, NSLOT - s)
        runs.append((s, r, n))
        r += n
    return runs


def _row_pieces(a):
    """Matmul N-pieces for block-row a: [(i0, ni), ...] respecting the
    rolling-slot wrap and the 504-column PSUM bank split."""
    s0 = a % NSLOT
    w = NSLOT - s0
    runs = [(0, 28)] if w >= 28 else [(0, w), (w, 28 - w)]
    pieces = []
    for i0, n in runs:
        end = i0 + n
        for b0, b1 in ((0, 14), (14, 28)):
            lo, hi = max(i0, b0), min(end, b1)
            if lo < hi:
                pieces.append((lo, hi - lo))
    return pieces


def _build_nc(debug=False, waitsplit=True):
    nc = bass.Bass()
    in1_d = nc.dram_tensor("in1", [C, H, W], DT.float32, kind="ExternalInput")
    in2_d = nc.dram_tensor("in2", [C, H, W], DT.float32, kind="ExternalInput")
    out_d = nc.dram_tensor("out", [ND, H, W], DT.float32, kind="ExternalOutput")
    dbg = {}
    if debug:
        dbg["bands"] = nc.dram_tensor(
            "dbg_bands", [128, BANDS_F], DT.bfloat16, kind="ExternalOutput"
        )
        dbg["in1blk"] = nc.dram_tensor(
            "dbg_in1blk", [128, IN1BLK_F], DT.bfloat16, kind="ExternalOutput"
        )
        dbg["band_sb"] = nc.dram_tensor(
            "dbg_band_sb", [128, FB], DT.float32, kind="ExternalOutput"
        )
        dbg["alig"] = nc.dram_tensor(
            "dbg_alig", [128, ND], DT.float32, kind="ExternalOutput"
        )
        dbg["tr"] = nc.dram_tensor(
            "dbg_tr", [128, 512], DT.float32, kind="ExternalOutput"
        )
        dbg["bands2"] = nc.dram_tensor(
            "dbg_bands2", [128, BANDS_F], DT.bfloat16, kind="ExternalOutput"
        )
        dbg["band_sb2"] = nc.dram_tensor(
            "dbg_band_sb2", [128, FB], DT.float32, kind="ExternalOutput"
        )
        dbg["alig2"] = nc.dram_tensor(
            "dbg_alig2", [128, ND], DT.float32, kind="ExternalOutput"
        )

    with TileContext(nc) as tc:
        with (
            tc.tile_pool(name="constp", bufs=1) as constp,
            tc.tile_pool(name="bandsp", bufs=1) as bandsp,
            tc.tile_pool(name="stgp", bufs=2) as stgp,
            tc.tile_pool(name="in1p", bufs=2) as in1p,
            tc.tile_pool(name="bandsbp", bufs=2) as bandsbp,
            tc.tile_pool(name="aligp", bufs=3) as aligp,
            tc.tile_pool(name="outp", bufs=1) as outp,
            tc.tile_pool(name="relup", bufs=2) as relup,
            tc.tile_pool(name="psp", bufs=2, space="PSUM") as psp,
            tc.tile_pool(name="trpp", bufs=2, space="PSUM") as trpp,
            tc.tile_pool(name="dramp", bufs=4, space="DRAM") as dramp,
        ):
            identity = constp.tile([128, 128], DT.float32)
            make_identity(nc, identity)

            bands = constp.tile([128, BANDS_F], DT.bfloat16)

            # zero everything once: covers w-padding columns and all
            # initial padding rows (contiguous write = precise dep tracking)
            nc.vector.memset(bands[:, :], 0.0)

            def build_group(g):
                """Fill band rows for padded parity rows [8g, 8g+8)."""
                glo, ghi = 8 * g, min(8 * g + 8, HEP)
                # zero spans (padding rows) - skip for g<2 (initial memset
                # covered them); needed when slots are being recycled
                for zlo, zhi in ((glo, min(ghi, 10)), (max(glo, 58), ghi)):
                    if zlo >= zhi or zhi <= NSLOT:
                        continue
                    for s0, _, n in _slot_runs(zlo, zhi):
                        for cc in range(CC):
                            for hp in range(2):
                                for wp in range(2):
                                    for wb in range(4):
                                        nc.vector.memset(
                                            _ap(
                                                bands,
                                                _band_base(cc, hp, wp, wb)
                                                + s0 * WIN_W,
                                                [[BANDS_F, 128], [1, n * WIN_W]],
                                            ),
                                            0.0,
                                        )
                # data span
                dlo, dhi = max(glo, 10), min(ghi, 58)
                if dlo >= dhi:
                    return
                h0, nh = 2 * (dlo - 10), 2 * (dhi - dlo)
                stg = stgp.tile([128, STG_F], DT.bfloat16, name="stg", bufs=1)
                for cc in range(CC):
                    nc.gpsimd.dma_start(
                        _ap(stg, cc * 16 * W, [[STG_F, 128], [1, nh * W]]),
                        in2_d[cc * 128 : (cc + 1) * 128, h0 : h0 + nh, :],
                    )
                for cc in range(CC):
                    for hp in range(2):
                        for wp in range(2):
                            for s0, he0, n in _slot_runs(dlo, dhi):
                                src_r = 2 * (he0 - dlo) + hp
                                for wb in range(4):
                                    # valid u range for this wb (w-padding)
                                    u0 = 10 if wb == 0 else 0
                                    u1 = 26 if wb == 3 else WIN_W
                                    nu = u1 - u0
                                    s_ap = _ap(
                                        stg,
                                        cc * 16 * W
                                        + src_r * W
                                        + (2 * (16 * wb + u0 - 10) + wp),
                                        [[STG_F, 128], [2 * W, n], [2, nu]],
                                    )
                                    d_ap = _ap(
                                        bands,
                                        _band_base(cc, hp, wp, wb)
                                        + s0 * WIN_W
                                        + u0,
                                        [[BANDS_F, 128], [WIN_W, n], [1, nu]],
                                    )
                                    nc.gpsimd.tensor_copy(d_ap, s_ap)

            def build_in1_slab(ai):
                """Parity-blocked in1 for block-row ai -> [128, IN1BLK_F]."""
                stg1 = stgp.tile([128, STG_F], DT.bfloat16, name="stg1", bufs=1)
                for cc in range(CC):
                    nc.gpsimd.dma_start(
                        _ap(stg1, cc * 16 * W, [[STG_F, 128], [1, 16 * W]]),
                        in1_d[cc * 128 : (cc + 1) * 128, 16 * ai : 16 * ai + 16, :],
                    )
                blk = in1p.tile([128, IN1BLK_F], DT.bfloat16, name="in1blk")
                for cc in range(CC):
                    for hp in range(2):
                        for wp in range(2):
                            src = _ap(
                                stg1,
                                cc * 16 * W + hp * W + wp,
                                [[STG_F, 128], [32, 4], [2 * W, HB], [2, WB]],
                            )
                            dst = _ap(
                                blk,
                                ((cc * 2 + hp) * 2 + wp) * 512,
                                [[IN1BLK_F, 128], [128, 4], [16, HB], [1, WB]],
                            )
                            nc.gpsimd.tensor_copy(dst, src)
                return blk

            # prologue: band groups 0-3, first in1 slab
            for g in range(4):
                build_group(g)
            in1blk = build_in1_slab(0)
            if debug:
                nc.sync.dma_start(
                    bass.AP(tensor=dbg["bands"], offset=0, ap=[[BANDS_F, 128], [1, BANDS_F]]),
                    bands[:, :],
                )
                nc.sync.dma_start(
                    bass.AP(tensor=dbg["in1blk"], offset=0, ap=[[IN1BLK_F, 128], [1, IN1BLK_F]]),
                    in1blk[:, :],
                )

            for k, a in enumerate(range(0, HE, HB)):  # 6 block-rows
                out_t = [
                    outp.tile([128, 16 * W], DT.float32, name=f"outt{dc}")
                    for dc in range(4)
                ]
                pieces = _row_pieces(a)
                for hp in range(2):
                    for wp in range(2):
                        for wb in range(4):
                            ps_pieces = [
                                psp.tile([128, 504], DT.float32, name="ps_a"),
                                psp.tile([128, 504], DT.float32, name="ps_b"),
                            ]
                            # one PSUM accumulation group per bank: start
                            # only on the bank's first write, stop on its last
                            bank_pieces = {0: [], 1: []}
                            for i0, ni in pieces:
                                bank_pieces[0 if i0 < 14 else 1].append((i0, ni))
                            for cc in range(CC):
                                lhsT = _ap(
                                    in1blk,
                                    (((cc * 2 + hp) * 2 + wp) * 4 + wb) * 128,
                                    [[IN1BLK_F, 128], [1, 128]],
                                )
                                for i0, ni in pieces:
                                    s0 = (a + i0) % NSLOT
                                    rhs = _ap(
                                        bands,
                                        _band_base(cc, hp, wp, wb) + s0 * WIN_W,
                                        [[BANDS_F, 128], [1, ni * WIN_W]],
                                    )
                                    pi = 0 if i0 < 14 else 1
                                    c0 = (i0 - 14 * pi) * WIN_W
                                    bp = bank_pieces[pi]
                                    nc.tensor.matmul(
                                        ps_pieces[pi][:, c0 : c0 + ni * WIN_W],
                                        lhsT,
                                        rhs,
                                        start=(cc == 0 and (i0, ni) == bp[0]),
                                        stop=(cc == CC - 1 and (i0, ni) == bp[-1]),
                                    )
                            # band -> SBUF -> DRAM
                            band_sb = bandsbp.tile([128, FB], DT.float32, name="band_sb")
                            nc.vector.tensor_copy(band_sb[:, 0:504], ps_pieces[0][:, :])
                            nc.vector.tensor_copy(
                                band_sb[:, 504:1008], ps_pieces[1][:, :]
                            )
                            if debug and (a, hp, wp, wb) == (0, 0, 0, 0):
                                nc.sync.dma_start(
                                    bass.AP(tensor=dbg["band_sb"], offset=0, ap=[[FB, 128], [1, FB]]),
                                    band_sb[:, :],
                                )
                            if debug and (a, hp, wp, wb) == (16, 0, 0, 0):
                                nc.sync.dma_start(
                                    bass.AP(tensor=dbg["band_sb2"], offset=0, ap=[[FB, 128], [1, FB]]),
                                    band_sb[:, :],
                                )
                            bdram = dramp.tile([128, FB], DT.float32, name="bdram")
                            nc.sync.dma_start(bdram[:, :], band_sb[:, :])
                            # diagonal gather DRAM -> ALIGNED[pixel, 441]
                            alig = aligp.tile([128, ND], DT.float32, name="alig")
                            for he in range(HB):
                                src = _ap(
                                    bdram,
                                    he * (16 * FB + WIN_W),
                                    [[FB + 1, 16], [WIN_W, NP], [1, NP]],
                                )
                                dst = _ap(
                                    alig,
                                    he * 16 * ND,
                                    [[ND, 16], [NP, NP], [1, NP]],
                                )
                                eng = nc.sync if he % 2 == 0 else nc.scalar
                                eng.dma_start(dst, src)
                            if debug and (a, hp, wp, wb) == (0, 0, 0, 0):
                                nc.sync.dma_start(
                                    bass.AP(tensor=dbg["alig"], offset=0, ap=[[ND, 128], [1, ND]]),
                                    alig[:, :],
                                )
                            if debug and (a, hp, wp, wb) == (16, 0, 0, 0):
                                nc.sync.dma_start(
                                    bass.AP(tensor=dbg["alig2"], offset=0, ap=[[ND, 128], [1, ND]]),
                                    alig[:, :],
                                )
                            # transpose pixel-major -> d-major
                            tr = trpp.tile([128, 512], DT.float32, name="tr")
                            for dc in range(4):
                                dlo = dc * 128
                                nd = min(128, ND - dlo)
                                nc.tensor.transpose(
                                    tr[0:nd, dc * 128 : dc * 128 + 128],
                                    alig[:, dlo : dlo + nd],
                                    identity[:, :],
                                )
                            if debug and (a, hp, wp, wb) == (0, 0, 0, 0):
                                nc.vector.tensor_copy(band_sb[:, 0:512], tr[:, :])
                                nc.sync.dma_start(
                                    bass.AP(tensor=dbg["tr"], offset=0, ap=[[512, 128], [1, 512]]),
                                    band_sb[:, 0:512],
                                )
                            # epilogue: leaky(x/C) = 0.1*x/C + relu(0.9*x/C)
                            relu_sb = relup.tile([128, 512], DT.float32, name="relu_sb")
                            for dc in range(4):
                                dlo = dc * 128
                                nd = min(128, ND - dlo)
                                nc.scalar.activation(
                                    relu_sb[0:nd, dc * 128 : dc * 128 + 128],
                                    tr[0:nd, dc * 128 : dc * 128 + 128],
                                    mybir.ActivationFunctionType.Relu,
                                    bias=0.0,
                                    scale=0.9 / C,
                                )
                            for dc in range(4):
                                dlo = dc * 128
                                nd = min(128, ND - dlo)
                                t_ap = _ap(
                                    tr, dc * 128, [[512, nd], [16, HB], [1, WB]]
                                )
                                r_ap = _ap(
                                    relu_sb, dc * 128, [[512, nd], [16, HB], [1, WB]]
                                )
                                dst = _ap(
                                    out_t[dc],
                                    hp * W + 32 * wb + wp,
                                    [[16 * W, nd], [2 * W, HB], [2, WB]],
                                )
                                nc.vector.scalar_tensor_tensor(
                                    dst,
                                    t_ap,
                                    0.1 / C,
                                    r_ap,
                                    mybir.AluOpType.mult,
                                    mybir.AluOpType.add,
                                )
                # prefetch next band group / in1 slab
                if 4 + k < 9:
                    build_group(4 + k)
                if k + 1 < 6:
                    in1blk = build_in1_slab(k + 1)
                # stores for this block-row
                for dc in range(4):
                    dlo = dc * 128
                    nd = min(128, ND - dlo)
                    dst = bass.AP(
                        tensor=out_d,
                        offset=dlo * HW + (2 * a) * W,
                        ap=[[HW, nd], [1, 16 * W]],
                    )
                    nc.sync.dma_start(dst, out_t[dc][0:nd, :])
            if debug:
                nc.sync.dma_start(
                    bass.AP(tensor=dbg["bands2"], offset=0, ap=[[BANDS_F, 128], [1, BANDS_F]]),
                    bands[:, :],
                )

    if waitsplit:
        _split_excess_waits(nc)
    return nc


_NC_CACHE = None


def _get_nc():
    global _NC_CACHE
    if _NC_CACHE is None:
        _NC_CACHE = _build_nc()
    return _NC_CACHE


def kernel(input1, input2):
    input1 = np.ascontiguousarray(np.asarray(input1, dtype=np.float32))
    input2 = np.ascontiguousarray(np.asarray(input2, dtype=np.float32))
    assert input1.shape == (B, C, H, W) and input2.shape == (B, C, H, W)
    nc = _get_nc()
    in_maps = [{"in1": input1[b], "in2": input2[b]} for b in range(B)]
    res = run_bass_kernel_spmd(nc, in_maps, core_ids=list(range(B)))
    return np.stack([res.results[b]["out"] for b in range(B)], axis=0)



# revision 28
# speedup vs baseline: 1.8567x; 1.8567x over previous
"""Trainium2 Bass kernel for nn_Correlation (FlowNet-style cost volume).

Problem: input1/input2 [8, 256, 96, 128] f32 ->
         out [8, 441, 96, 128] f32
  out[b, 21*i+j, h, w] = leaky_relu_0.1( (1/256) * sum_c
        in1[b,c,h,w] * in2pad[b,c, h+2i, w+2j] )       (pad 20 each side)

Strategy (data-parallel over B across 8 cores; per core = 1 sample):
  * Displacements are even (dilation 2): pixel (h,w) only correlates with
    in2 pixels of the same (h%2, w%2) parity class. Per parity class the
    dilated 21x21 patch is a dense 21x21 window over the 48x64 parity
    image.
  * in2 is kept as 8 resident padded parity-class images [128c, 68x84]
    bf16 in SBUF (pad 10 each side), built by large strided copies from
    chunked contiguous cast-DMA loads; the matmul ifmap reads 28x36
    windows directly via strided APs (no band duplication).
  * in1: per block-row, row-major slabs are rearranged once into
    parity-blocked lhsT tiles (one 4-dim copy per class) so matmul
    weights APs are single-free-dim.
  * Per block (8he x 16we pixels = 128 partitions): 2x2 matmuls contract
    C=256 over the 28x36 window -> PSUM band [pix, 1008]. ScalarE
    applies leaky_relu(x/C) (fused Lrelu w/ alpha) during PSUM->SBUF
    evacuation in bf16.
  * Per-pixel alignment (441 of 1008 cols, per-pixel diagonal offset) is
    done via a DRAM bounce: one contiguous write + one 3-dim diagonal
    gather read (flat DRAM-side APs make the diagonal legal). bf16 both
    ways; one DMA each.
  * PE transposes the aligned [pix, 441] (strided from the 741-wide
    gather stage) to d-major via identity matmuls; DVE copies scatter
    into a parity-interleaved bf16 out tile; gpsimd cast-DMA stores
    fp32.
  * Software-pipelined emission (write k, read k-2, transpose k-3) so
    in-order engine queues never block on in-flight DMAs.
"""

import numpy as np

import concourse.bass as bass
import concourse.mybir as mybir
from concourse.tile import TileContext
from concourse.bass_utils import run_bass_kernel_spmd
from concourse.masks import make_identity

DT = mybir.dt
AF = mybir.ActivationFunctionType

# ---- problem geometry ----
B, C, H, W = 8, 256, 96, 128
NP = 21                      # displacements per axis
ND = NP * NP                 # 441
CC = 2                       # C chunks of 128
HE, WE = H // 2, W // 2      # parity image 48 x 64
PAD = 10                     # parity-unit halo (= MAX_DISP/2)
HEP, WEP = HE + 2 * PAD, WE + 2 * PAD  # 68 x 84
CLS_F = HEP * WEP            # 5712

HB, WB = 8, 16               # pixel block (he, we); HB*WB = 128
NWB = WE // WB               # 4 w-blocks
WIN_H, WIN_W = HB + 20, WB + 20   # 28 x 36
FB = WIN_H * WIN_W           # 1008 band columns
SPAN = WIN_W * 20 + 20 + 1   # 741: per-pixel gather span
HW = H * W                   # 12288
NBLK = HE // HB              # 6 block-rows
OT_F = 4 * 2 * HB * W        # 8192: out tile free size (4 d-chunks x 16 rows)

_MAX_WAITS = 1


def _split_excess_waits(nc):
    """This walrus build accepts only ONE sync-wait per instruction; Tile
    emits multi-waits. Hoist excess waits onto same-engine NOPs inserted
    right before the over-subscribed instruction."""
    nid = 0
    for f in nc.m.functions:
        for blk in f.blocks:
            insts = list(blk.instructions)
            out = []
            changed = False
            for inst in insts:
                si = inst.sync_info
                if si is not None and si.on_wait and len(si.on_wait) > _MAX_WAITS:
                    waits = list(si.on_wait)
                    extra, keep = waits[:-_MAX_WAITS], waits[-_MAX_WAITS:]
                    for k in range(0, len(extra), _MAX_WAITS):
                        nop = mybir.InstNoOp(name=f"I-waitsplit-{nid}", ins=[], outs=[])
                        nid += 1
                        nop.engine = inst.engine
                        nop.sync_info = mybir.SyncInfo(
                            on_wait=extra[k : k + _MAX_WAITS], on_update=[]
                        )
                        out.append(nop)
                        changed = True
                    si.on_wait = keep
                    inst.sync_info = si
                out.append(inst)
            if changed:
                blk.instructions = out
    return nc


def _ap(t, off_extra, dims):
    return bass.AP(tensor=t.tensor, offset=t.offset + off_extra, ap=dims)


def _build_nc(debug=False, waitsplit=True):
    nc = bass.Bass()
    in1_d = nc.dram_tensor("in1", [C, H, W], DT.float32, kind="ExternalInput")
    in2_d = nc.dram_tensor("in2", [C, H, W], DT.float32, kind="ExternalInput")
    out_d = nc.dram_tensor("out", [ND, H, W], DT.float32, kind="ExternalOutput")
    dbg = {}
    if debug:
        dbg["band"] = nc.dram_tensor(
            "dbg_band", [128, FB], DT.bfloat16, kind="ExternalOutput"
        )
        dbg["stage"] = nc.dram_tensor(
            "dbg_stage", [128, SPAN], DT.bfloat16, kind="ExternalOutput"
        )

    with TileContext(nc) as tc:
        with (
            tc.tile_pool(name="constp", bufs=1) as constp,
            tc.tile_pool(name="slabp", bufs=1) as slabp,
            tc.tile_pool(name="lhsp", bufs=2) as lhsp,
            tc.tile_pool(name="stgp", bufs=2) as stgp,
            tc.tile_pool(name="bsbp", bufs=3) as bsbp,
            tc.tile_pool(name="stagep", bufs=4) as stagep,
            tc.tile_pool(name="aligp", bufs=3) as aligp,
            tc.tile_pool(name="relp", bufs=2) as relp,
            tc.tile_pool(name="outp", bufs=2) as outp,
            tc.tile_pool(name="psp", bufs=3, space="PSUM") as psp,
            tc.tile_pool(name="trpp", bufs=2, space="PSUM") as trpp,
            tc.tile_pool(name="dramp", bufs=4, space="DRAM") as dramp,
        ):
            identity = constp.tile([128, 128], DT.bfloat16)
            make_identity(nc, identity)
            alpha_t = constp.tile([128, 1], DT.float32, name="alpha01")
            nc.vector.memset(alpha_t[:, :], 0.1)

            cls = {}
            for cc in range(CC):
                for hp in range(2):
                    for wp in range(2):
                        cls[cc, hp, wp] = constp.tile(
                            [128, CLS_F], DT.bfloat16, name=f"cls{cc}{hp}{wp}"
                        )

            # zero the padding halos (data region is overwritten by loads)
            for ti, t in enumerate(cls.values()):
                me = nc.vector if ti % 2 == 0 else nc.gpsimd
                me.memset(_ap(t, 0, [[CLS_F, 128], [1, PAD * WEP]]), 0.0)
                me.memset(
                    _ap(t, (HEP - PAD) * WEP, [[CLS_F, 128], [1, PAD * WEP]]), 0.0
                )
                me.memset(_ap(t, PAD * WEP, [[CLS_F, 128], [WEP, HE], [1, PAD]]), 0.0)
                me.memset(
                    _ap(t, PAD * WEP + PAD + WE, [[CLS_F, 128], [WEP, HE], [1, PAD]]),
                    0.0,
                )

            def load_slab(ab):
                """in1 rows [16ab, 16ab+16) as bf16, row-major."""
                slabs = []
                for cc in range(CC):
                    s = slabp.tile([128, 2 * HB * W], DT.bfloat16, name=f"slab{cc}")
                    nc.gpsimd.dma_start(
                        s[:, :],
                        in1_d[cc * 128 : (cc + 1) * 128, 16 * ab : 16 * ab + 16, :],
                    )
                    slabs.append(s)
                return slabs

            def rearrange_lhs(ab, slabs):
                """slab -> lhs[cc,hp,wp]: [128c, wb*128 + 16he + we] (bf16).

                One 4-dim copy per class; round-robin engines."""
                # scale by 1/C here so the PE output needs no rescale
                eng = [lambda d, s: nc.vector.tensor_scalar_mul(d, s, 1.0 / C)]
                lhs = {}
                n = 0
                for cc in range(CC):
                    for hp in range(2):
                        for wp in range(2):
                            t = lhsp.tile(
                                [128, NWB * 128], DT.bfloat16, name=f"lhs{cc}{hp}{wp}"
                            )
                            src = _ap(
                                slabs[cc],
                                hp * W + wp,
                                [[2 * HB * W, 128], [2 * WB, NWB], [2 * W, HB], [2, WB]],
                            )
                            dst = _ap(
                                t,
                                0,
                                [[NWB * 128, 128], [128, NWB], [WB, HB], [1, WB]],
                            )
                            eng[0](dst, src)
                            n += 1
                            lhs[cc, hp, wp] = t
                return lhs

            slabs = load_slab(0)
            lhs_cur = rearrange_lhs(0, slabs)
            lhs_next = None

            # in2 ingest: 8 chunks of 12 full-res rows, pipelined
            prev = None

            def split_chunk(k6, stg):
                eng = [
                    lambda d, s: nc.scalar.copy(d, s),
                    lambda d, s: nc.scalar.copy(d, s),
                ]
                n = 0
                for cc in range(CC):
                    for hp in range(2):
                        for wp in range(2):
                            src = _ap(
                                stg,
                                cc * 1536 + hp * W + wp,
                                [[3072, 128], [2 * W, 6], [2, WE]],
                            )
                            dst = _ap(
                                cls[cc, hp, wp],
                                (PAD + 6 * k6) * WEP + PAD,
                                [[CLS_F, 128], [WEP, 6], [1, WE]],
                            )
                            eng[n % 2](dst, src)
                            n += 1

            for k6 in range(8):
                stg = stgp.tile([128, 3072], DT.bfloat16, name="stg")
                for cc in range(CC):
                    nc.gpsimd.dma_start(
                        _ap(stg, cc * 1536, [[3072, 128], [1, 1536]]),
                        in2_d[cc * 128 : (cc + 1) * 128, 12 * k6 : 12 * k6 + 12, :],
                    )
                if prev is not None:
                    split_chunk(k6 - 1, prev)
                prev = stg
            split_chunk(7, prev)

            bands = [
                (ab, hp, wp, wb)
                for ab in range(NBLK)
                for hp in range(2)
                for wp in range(2)
                for wb in range(NWB)
            ]
            n = len(bands)
            nbr = 2 * 2 * NWB  # bands per block-row (16)
            bdrams = {}
            stages = {}
            out_t = None

            def mm(k):
                nonlocal slabs, lhs_cur, lhs_next
                ab, hp, wp, wb = bands[k]
                if k % nbr == 0 and ab + 1 < NBLK:
                    slabs = load_slab(ab + 1)
                if k % nbr == nbr // 2 and ab + 1 < NBLK:
                    lhs_next = rearrange_lhs(ab + 1, slabs)
                a = HB * ab
                # pieces at cols 0 and 512 so each stays inside one PSUM bank
                ps = psp.tile([128, 1024], DT.float32, name="ps")
                for cc in range(CC):
                    lhsT = _ap(
                        lhs_cur[cc, hp, wp],
                        wb * 128,
                        [[NWB * 128, 128], [1, 128]],
                    )
                    for t in range(2):
                        rhs = _ap(
                            cls[cc, hp, wp],
                            (a + 14 * t) * WEP + WB * wb,
                            [[CLS_F, 128], [WEP, 14], [1, WIN_W]],
                        )
                        nc.tensor.matmul(
                            ps[:, 512 * t : 512 * t + 504],
                            lhsT,
                            rhs,
                            start=(cc == 0),
                            stop=(cc == CC - 1),
                        )
                # evacuate PSUM (cast to bf16; leaky applied post-transpose)
                bsb = bsbp.tile([128, 1024], DT.bfloat16, name="bsb")
                nc.scalar.copy(bsb[:, :], ps[:, :])
                if debug and k == 0:
                    nc.sync.dma_start(
                        bass.AP(tensor=dbg["band"], offset=0, ap=[[FB, 128], [1, FB]]),
                        bsb[:, :],
                    )
                bd = dramp.tile([128, FB], DT.bfloat16, name="bd")
                nc.sync.dma_start(
                    bd[:, :],
                    _ap(bsb, 0, [[1024, 128], [512, 2], [1, 504]]),
                )
                bdrams[k] = bd
                if k % nbr == nbr - 1 and lhs_next is not None:
                    lhs_cur, lhs_next = lhs_next, None

            def rd(k):
                bd = bdrams.pop(k)
                st = stagep.tile([128, SPAN], DT.bfloat16, name="stage")
                nc.sync.dma_start(
                    st[:, :],
                    _ap(bd, 0, [[WB * FB + WIN_W, HB], [FB + 1, WB], [1, SPAN]]),
                )
                stages[k] = st
                if debug and k == 0:
                    nc.scalar.dma_start(
                        bass.AP(
                            tensor=dbg["stage"], offset=0, ap=[[SPAN, 128], [1, SPAN]]
                        ),
                        st[:, :],
                    )

            def tp(k):
                nonlocal out_t
                ab, hp, wp, wb = bands[k]
                if k % nbr == 0:
                    out_t = outp.tile([128, OT_F], DT.bfloat16, name="ot")
                st = stages.pop(k)
                # unfold: alig[p, 21i+j] = stage[p, 36i+j] (dense 441 cols)
                alig = aligp.tile([128, ND], DT.bfloat16, name="alig")
                nc.scalar.copy(
                    _ap(alig, 0, [[ND, 128], [NP, NP], [1, NP]]),
                    _ap(st, 0, [[SPAN, 128], [WIN_W, NP], [1, NP]]),
                )
                tr = trpp.tile([128, 512], DT.float32, name="tr")
                for dc in range(4):
                    nd = min(128, ND - 128 * dc)
                    nc.tensor.matmul(
                        tr[0:nd, 128 * dc : 128 * dc + 128],
                        alig[:, 128 * dc : 128 * dc + nd],
                        identity[:, :],
                        start=True,
                        stop=True,
                    )
                # leaky: out = 0.1*x + max(0.9*x, 0) computed as relu-part (DVE
                # tensor_scalar) + stt combine during the out_t scatter
                rel = relp.tile([128, 512], DT.bfloat16, name="rel")
                nc.vector.tensor_scalar(
                    rel[:, :],
                    tr[:, :],
                    0.9,
                    0.0,
                    mybir.AluOpType.mult,
                    mybir.AluOpType.max,
                )
                base = hp * W + 2 * WB * wb + wp
                nc.vector.scalar_tensor_tensor(
                    _ap(
                        out_t,
                        base,
                        [[OT_F, 128], [2 * HB * W, 3], [2 * W, HB], [2, WB]],
                    ),
                    _ap(tr, 0, [[512, 128], [128, 3], [WB, HB], [1, WB]]),
                    0.1,
                    _ap(rel, 0, [[512, 128], [128, 3], [WB, HB], [1, WB]]),
                    mybir.AluOpType.mult,
                    mybir.AluOpType.add,
                )
                nc.vector.scalar_tensor_tensor(
                    _ap(
                        out_t,
                        3 * 2 * HB * W + base,
                        [[OT_F, 57], [2 * W, HB], [2, WB]],
                    ),
                    _ap(tr, 3 * 128, [[512, 57], [WB, HB], [1, WB]]),
                    0.1,
                    _ap(rel, 3 * 128, [[512, 57], [WB, HB], [1, WB]]),
                    mybir.AluOpType.mult,
                    mybir.AluOpType.add,
                )
                if k % nbr == nbr - 1:
                    nc.gpsimd.dma_start(
                        bass.AP(
                            tensor=out_d,
                            offset=2 * HB * ab * W,
                            ap=[[HW, 128], [128 * HW, 3], [1, 2 * HB * W]],
                        ),
                        _ap(out_t, 0, [[OT_F, 128], [2 * HB * W, 3], [1, 2 * HB * W]]),
                    )
                    nc.gpsimd.dma_start(
                        bass.AP(
                            tensor=out_d,
                            offset=384 * HW + 2 * HB * ab * W,
                            ap=[[HW, 57], [1, 2 * HB * W]],
                        ),
                        _ap(out_t, 3 * 2 * HB * W, [[OT_F, 57], [1, 2 * HB * W]]),
                    )

            for k in range(n + 4):
                if k < n:
                    mm(k)
                if 0 <= k - 2 < n:
                    rd(k - 2)
                if 0 <= k - 4 < n:
                    tp(k - 4)

    if waitsplit:
        _split_excess_waits(nc)
    return nc


_NC_CACHE = None


def _get_nc():
    global _NC_CACHE
    if _NC_CACHE is None:
        _NC_CACHE = _build_nc()
    return _NC_CACHE


def kernel(input1, input2):
    input1 = np.ascontiguousarray(np.asarray(input1, dtype=np.float32))
    input2 = np.ascontiguousarray(np.asarray(input2, dtype=np.float32))
    assert input1.shape == (B, C, H, W) and input2.shape == (B, C, H, W)
    nc = _get_nc()
    in_maps = [{"in1": input1[b], "in2": input2[b]} for b in range(B)]
    res = run_bass_kernel_spmd(nc, in_maps, core_ids=list(range(B)))
    return np.stack([res.results[b]["out"] for b in range(B)], axis=0)


# revision 31
# speedup vs baseline: 2.1292x; 1.1468x over previous
"""Trainium2 Bass kernel for nn_Correlation (FlowNet-style cost volume).

Problem: input1/input2 [8, 256, 96, 128] f32 ->
         out [8, 441, 96, 128] f32
  out[b, 21*i+j, h, w] = leaky_relu_0.1( (1/256) * sum_c
        in1[b,c,h,w] * in2pad[b,c, h+2i, w+2j] )       (pad 20 each side)

Strategy (data-parallel over B across 8 cores; per core = 1 sample):
  * Displacements are even (dilation 2): pixel (h,w) only correlates with
    in2 pixels of the same (h%2, w%2) parity class. Per parity class the
    dilated 21x21 patch is a dense 21x21 window over the 48x64 parity
    image.
  * in2 is kept as 8 resident padded parity-class images [128c, 68x84]
    bf16 in SBUF (pad 10 each side), built by large strided copies from
    chunked contiguous cast-DMA loads; the matmul ifmap reads 28x36
    windows directly via strided APs (no band duplication).
  * in1: per block-row, row-major slabs are rearranged once into
    parity-blocked lhsT tiles (one 4-dim copy per class) so matmul
    weights APs are single-free-dim.
  * Per block (8he x 16we pixels = 128 partitions): 2x2 matmuls contract
    C=256 over the 28x36 window -> PSUM band [pix, 1008]. ScalarE
    applies leaky_relu(x/C) (fused Lrelu w/ alpha) during PSUM->SBUF
    evacuation in bf16.
  * Per-pixel alignment (441 of 1008 cols, per-pixel diagonal offset) is
    done via a DRAM bounce: one contiguous write + one 3-dim diagonal
    gather read (flat DRAM-side APs make the diagonal legal). bf16 both
    ways; one DMA each.
  * PE transposes the aligned [pix, 441] (strided from the 741-wide
    gather stage) to d-major via identity matmuls; DVE copies scatter
    into a parity-interleaved bf16 out tile; gpsimd cast-DMA stores
    fp32.
  * Software-pipelined emission (write k, read k-2, transpose k-3) so
    in-order engine queues never block on in-flight DMAs.
"""

import numpy as np

import concourse.bass as bass
import concourse.mybir as mybir
from concourse.tile import TileContext
from concourse.bass_utils import run_bass_kernel_spmd
from concourse.masks import make_identity

DT = mybir.dt
AF = mybir.ActivationFunctionType

# ---- problem geometry ----
B, C, H, W = 8, 256, 96, 128
NP = 21                      # displacements per axis
ND = NP * NP                 # 441
CC = 2                       # C chunks of 128
HE, WE = H // 2, W // 2      # parity image 48 x 64
PAD = 10                     # parity-unit halo (= MAX_DISP/2)
HEP, WEP = HE + 2 * PAD, WE + 2 * PAD  # 68 x 84
CLS_F = HEP * WEP            # 5712

HB, WB = 8, 16               # pixel block (he, we); HB*WB = 128
NWB = WE // WB               # 4 w-blocks
WIN_H, WIN_W = HB + 20, WB + 20   # 28 x 36
FB = WIN_H * WIN_W           # 1008 band columns
SPAN = WIN_W * 20 + 20 + 1   # 741: per-pixel gather span
HW = H * W                   # 12288
NBLK = HE // HB              # 6 block-rows
OT_F = 4 * 2 * HB * W        # 8192: out tile free size (4 d-chunks x 16 rows)

_MAX_WAITS = 1


def _split_excess_waits(nc):
    """This walrus build accepts only ONE sync-wait per instruction; Tile
    emits multi-waits. Hoist excess waits onto same-engine NOPs inserted
    right before the over-subscribed instruction."""
    nid = 0
    for f in nc.m.functions:
        for blk in f.blocks:
            insts = list(blk.instructions)
            out = []
            changed = False
            for inst in insts:
                si = inst.sync_info
                if si is not None and si.on_wait and len(si.on_wait) > _MAX_WAITS:
                    waits = list(si.on_wait)
                    extra, keep = waits[:-_MAX_WAITS], waits[-_MAX_WAITS:]
                    for k in range(0, len(extra), _MAX_WAITS):
                        nop = mybir.InstNoOp(name=f"I-waitsplit-{nid}", ins=[], outs=[])
                        nid += 1
                        nop.engine = inst.engine
                        nop.sync_info = mybir.SyncInfo(
                            on_wait=extra[k : k + _MAX_WAITS], on_update=[]
                        )
                        out.append(nop)
                        changed = True
                    si.on_wait = keep
                    inst.sync_info = si
                out.append(inst)
            if changed:
                blk.instructions = out
    return nc


def _ap(t, off_extra, dims):
    return bass.AP(tensor=t.tensor, offset=t.offset + off_extra, ap=dims)


def _build_nc(debug=False, waitsplit=True):
    nc = bass.Bass()
    in1_d = nc.dram_tensor("in1", [C, H, W], DT.float32, kind="ExternalInput")
    in2_d = nc.dram_tensor("in2", [C, H, W], DT.float32, kind="ExternalInput")
    out_d = nc.dram_tensor("out", [ND, H, W], DT.float32, kind="ExternalOutput")
    dbg = {}
    if debug:
        dbg["band"] = nc.dram_tensor(
            "dbg_band", [128, FB], DT.bfloat16, kind="ExternalOutput"
        )
        dbg["stage"] = nc.dram_tensor(
            "dbg_stage", [128, SPAN], DT.bfloat16, kind="ExternalOutput"
        )

    with TileContext(nc) as tc:
        with (
            tc.tile_pool(name="constp", bufs=1) as constp,
            tc.tile_pool(name="slabp", bufs=2) as slabp,
            tc.tile_pool(name="lhsp", bufs=3) as lhsp,
            tc.tile_pool(name="stgp", bufs=2) as stgp,
            tc.tile_pool(name="bsbp", bufs=4) as bsbp,
            tc.tile_pool(name="stagep", bufs=5) as stagep,
            tc.tile_pool(name="aligp", bufs=4) as aligp,
            tc.tile_pool(name="relp", bufs=3) as relp,
            tc.tile_pool(name="outp", bufs=2) as outp,
            tc.tile_pool(name="psp", bufs=3, space="PSUM") as psp,
            tc.tile_pool(name="trpp", bufs=2, space="PSUM") as trpp,
            tc.tile_pool(name="dramp", bufs=6, space="DRAM") as dramp,
        ):
            identity = constp.tile([128, 128], DT.bfloat16)
            make_identity(nc, identity)
            alpha_t = constp.tile([128, 1], DT.float32, name="alpha01")
            nc.vector.memset(alpha_t[:, :], 0.1)

            cls = {}
            for cc in range(CC):
                for hp in range(2):
                    for wp in range(2):
                        cls[cc, hp, wp] = constp.tile(
                            [128, CLS_F], DT.bfloat16, name=f"cls{cc}{hp}{wp}"
                        )

            # zero the padding halos (data region is overwritten by loads)
            for ti, t in enumerate(cls.values()):
                me = nc.vector if ti % 2 == 0 else nc.gpsimd
                me.memset(_ap(t, 0, [[CLS_F, 128], [1, PAD * WEP]]), 0.0)
                me.memset(
                    _ap(t, (HEP - PAD) * WEP, [[CLS_F, 128], [1, PAD * WEP]]), 0.0
                )
                me.memset(_ap(t, PAD * WEP, [[CLS_F, 128], [WEP, HE], [1, PAD]]), 0.0)
                me.memset(
                    _ap(t, PAD * WEP + PAD + WE, [[CLS_F, 128], [WEP, HE], [1, PAD]]),
                    0.0,
                )

            def load_slab(ab):
                """in1 rows [16ab, 16ab+16) as bf16, row-major."""
                slabs = []
                for cc in range(CC):
                    s = slabp.tile([128, 2 * HB * W], DT.bfloat16, name=f"slab{cc}")
                    nc.gpsimd.dma_start(
                        s[:, :],
                        in1_d[cc * 128 : (cc + 1) * 128, 16 * ab : 16 * ab + 16, :],
                    )
                    slabs.append(s)
                return slabs

            def rearrange_lhs(ab, slabs):
                """slab -> lhs[cc,hp,wp]: [128c, wb*128 + 16he + we] (bf16).

                One 4-dim copy per class; round-robin engines."""
                # scale by 1/C here so the PE output needs no rescale
                eng = [lambda d, s: nc.vector.tensor_scalar_mul(d, s, 1.0 / C)]
                lhs = {}
                n = 0
                for cc in range(CC):
                    for hp in range(2):
                        for wp in range(2):
                            t = lhsp.tile(
                                [128, NWB * 128], DT.bfloat16, name=f"lhs{cc}{hp}{wp}"
                            )
                            src = _ap(
                                slabs[cc],
                                hp * W + wp,
                                [[2 * HB * W, 128], [2 * WB, NWB], [2 * W, HB], [2, WB]],
                            )
                            dst = _ap(
                                t,
                                0,
                                [[NWB * 128, 128], [128, NWB], [WB, HB], [1, WB]],
                            )
                            eng[0](dst, src)
                            n += 1
                            lhs[cc, hp, wp] = t
                return lhs

            slabs = load_slab(0)
            lhs_cur = rearrange_lhs(0, slabs)
            lhs_next = None

            # in2 ingest: 8 chunks of 12 full-res rows, pipelined
            prev = None

            def split_chunk(k6, stg):
                eng = [
                    lambda d, s: nc.scalar.copy(d, s),
                    lambda d, s: nc.scalar.copy(d, s),
                ]
                n = 0
                for cc in range(CC):
                    for hp in range(2):
                        for wp in range(2):
                            src = _ap(
                                stg,
                                cc * 1536 + hp * W + wp,
                                [[3072, 128], [2 * W, 6], [2, WE]],
                            )
                            dst = _ap(
                                cls[cc, hp, wp],
                                (PAD + 6 * k6) * WEP + PAD,
                                [[CLS_F, 128], [WEP, 6], [1, WE]],
                            )
                            eng[n % 2](dst, src)
                            n += 1

            for k6 in range(8):
                stg = stgp.tile([128, 3072], DT.bfloat16, name="stg")
                for cc in range(CC):
                    nc.gpsimd.dma_start(
                        _ap(stg, cc * 1536, [[3072, 128], [1, 1536]]),
                        in2_d[cc * 128 : (cc + 1) * 128, 12 * k6 : 12 * k6 + 12, :],
                    )
                if prev is not None:
                    split_chunk(k6 - 1, prev)
                prev = stg
            split_chunk(7, prev)

            bands = [
                (ab, hp, wp, wb)
                for ab in range(NBLK)
                for hp in range(2)
                for wp in range(2)
                for wb in range(NWB)
            ]
            n = len(bands)
            nbr = 2 * 2 * NWB  # bands per block-row (16)
            bdrams = {}
            stages = {}
            out_t = None

            def mm(k):
                nonlocal slabs, lhs_cur, lhs_next
                ab, hp, wp, wb = bands[k]
                if k % nbr == 0 and ab + 1 < NBLK:
                    slabs = load_slab(ab + 1)
                if k % nbr == 2 and ab + 1 < NBLK:
                    lhs_next = rearrange_lhs(ab + 1, slabs)
                a = HB * ab
                # pieces at cols 0 and 512 so each stays inside one PSUM bank
                ps = psp.tile([128, 1024], DT.float32, name="ps")
                for cc in range(CC):
                    lhsT = _ap(
                        lhs_cur[cc, hp, wp],
                        wb * 128,
                        [[NWB * 128, 128], [1, 128]],
                    )
                    for t in range(2):
                        rhs = _ap(
                            cls[cc, hp, wp],
                            (a + 14 * t) * WEP + WB * wb,
                            [[CLS_F, 128], [WEP, 14], [1, WIN_W]],
                        )
                        nc.tensor.matmul(
                            ps[:, 512 * t : 512 * t + 504],
                            lhsT,
                            rhs,
                            start=(cc == 0),
                            stop=(cc == CC - 1),
                        )
                # evacuate PSUM (cast to bf16; leaky applied post-transpose)
                bsb = bsbp.tile([128, 1024], DT.bfloat16, name="bsb")
                nc.scalar.copy(bsb[:, :], ps[:, :])
                if debug and k == 0:
                    nc.sync.dma_start(
                        bass.AP(tensor=dbg["band"], offset=0, ap=[[FB, 128], [1, FB]]),
                        bsb[:, :],
                    )
                bd = dramp.tile([128, FB], DT.bfloat16, name="bd")
                nc.sync.dma_start(
                    bd[:, :],
                    _ap(bsb, 0, [[1024, 128], [512, 2], [1, 504]]),
                )
                bdrams[k] = bd
                if k % nbr == nbr - 1 and lhs_next is not None:
                    lhs_cur, lhs_next = lhs_next, None

            def rd(k):
                bd = bdrams.pop(k)
                st = stagep.tile([128, SPAN], DT.bfloat16, name="stage")
                nc.sync.dma_start(
                    st[:, :],
                    _ap(bd, 0, [[WB * FB + WIN_W, HB], [FB + 1, WB], [1, SPAN]]),
                )
                stages[k] = st
                if debug and k == 0:
                    nc.scalar.dma_start(
                        bass.AP(
                            tensor=dbg["stage"], offset=0, ap=[[SPAN, 128], [1, SPAN]]
                        ),
                        st[:, :],
                    )

            def tp(k):
                nonlocal out_t
                ab, hp, wp, wb = bands[k]
                if k % nbr == 0:
                    out_t = outp.tile([128, OT_F], DT.bfloat16, name="ot")
                st = stages.pop(k)
                # unfold: alig[p, 21i+j] = stage[p, 36i+j] (dense 441 cols)
                alig = aligp.tile([128, ND], DT.bfloat16, name="alig")
                ueng = (
                    (lambda d, s: nc.scalar.copy(d, s))
                    if k % 2 == 0
                    else (lambda d, s: nc.gpsimd.tensor_copy(d, s))
                )
                ueng(
                    _ap(alig, 0, [[ND, 128], [NP, NP], [1, NP]]),
                    _ap(st, 0, [[SPAN, 128], [WIN_W, NP], [1, NP]]),
                )
                tr = trpp.tile([128, 512], DT.float32, name="tr")
                for dc in range(4):
                    nd = min(128, ND - 128 * dc)
                    nc.tensor.matmul(
                        tr[0:nd, 128 * dc : 128 * dc + 128],
                        alig[:, 128 * dc : 128 * dc + nd],
                        identity[:, :],
                        start=True,
                        stop=True,
                    )
                # leaky: out = 0.1*x + max(0.9*x, 0) computed as relu-part (DVE
                # tensor_scalar) + stt combine during the out_t scatter
                rel = relp.tile([128, 512], DT.bfloat16, name="rel")
                nc.vector.tensor_scalar(
                    rel[:, :],
                    tr[:, :],
                    0.9,
                    0.0,
                    mybir.AluOpType.mult,
                    mybir.AluOpType.max,
                )
                base = hp * W + 2 * WB * wb + wp
                nc.vector.scalar_tensor_tensor(
                    _ap(
                        out_t,
                        base,
                        [[OT_F, 128], [2 * HB * W, 3], [2 * W, HB], [2, WB]],
                    ),
                    _ap(tr, 0, [[512, 128], [128, 3], [WB, HB], [1, WB]]),
                    0.1,
                    _ap(rel, 0, [[512, 128], [128, 3], [WB, HB], [1, WB]]),
                    mybir.AluOpType.mult,
                    mybir.AluOpType.add,
                )
                nc.vector.scalar_tensor_tensor(
                    _ap(
                        out_t,
                        3 * 2 * HB * W + base,
                        [[OT_F, 57], [2 * W, HB], [2, WB]],
                    ),
                    _ap(tr, 3 * 128, [[512, 57], [WB, HB], [1, WB]]),
                    0.1,
                    _ap(rel, 3 * 128, [[512, 57], [WB, HB], [1, WB]]),
                    mybir.AluOpType.mult,
                    mybir.AluOpType.add,
                )
                if k % nbr == nbr - 1:
                    nc.gpsimd.dma_start(
                        bass.AP(
                            tensor=out_d,
                            offset=2 * HB * ab * W,
                            ap=[[HW, 128], [128 * HW, 3], [1, 2 * HB * W]],
                        ),
                        _ap(out_t, 0, [[OT_F, 128], [2 * HB * W, 3], [1, 2 * HB * W]]),
                    )
                    nc.gpsimd.dma_start(
                        bass.AP(
                            tensor=out_d,
                            offset=384 * HW + 2 * HB * ab * W,
                            ap=[[HW, 57], [1, 2 * HB * W]],
                        ),
                        _ap(out_t, 3 * 2 * HB * W, [[OT_F, 57], [1, 2 * HB * W]]),
                    )

            for k in range(n + 4):
                if k < n:
                    mm(k)
                if 0 <= k - 2 < n:
                    rd(k - 2)
                if 0 <= k - 4 < n:
                    tp(k - 4)

    if waitsplit:
        _split_excess_waits(nc)
    return nc


_NC_CACHE = None


def _get_nc():
    global _NC_CACHE
    if _NC_CACHE is None:
        _NC_CACHE = _build_nc()
    return _NC_CACHE


def kernel(input1, input2):
    input1 = np.ascontiguousarray(np.asarray(input1, dtype=np.float32))
    input2 = np.ascontiguousarray(np.asarray(input2, dtype=np.float32))
    assert input1.shape == (B, C, H, W) and input2.shape == (B, C, H, W)
    nc = _get_nc()
    in_maps = [{"in1": input1[b], "in2": input2[b]} for b in range(B)]
    res = run_bass_kernel_spmd(nc, in_maps, core_ids=list(range(B)))
    return np.stack([res.results[b]["out"] for b in range(B)], axis=0)
